# revision 23
# baseline (speedup 1.0000x reference)
"""2-layer GAT (PyG-style) on TRN2, 8 NeuronCores.

Strategy: degree-bucketed node layout. Nodes sorted by in-degree (desc) and
dealt round-robin across the 8 cores into 49 tiles/core of 128 nodes each
(so every core's tile t has near-identical max degree -> one shared SPMD
schedule). Tiles are batched into GROUPS of j consecutive tiles sharing a
common (padded) degree d_g, so every per-edge DVE op runs once per group
instead of once per tile. Each group grid is [128 node-partitions x j x d_g]
edge slots padded with a sentinel row whose a_s = -1e30 (=> w = 0). Per-edge
source rows are fetched with dma_gather (GPSIMD bulk gather; signed int16
indices against mid-table base row 32768; chunked ~8 slots/call to keep the
descriptor rings draining across the 4 SWDGE queues).

Layer-1 table rows (bf16, 512B): [h (192, (c,h)-interleaved) | a_s 3 | a_d 3
| pad], all bf16. The exp() writes the edge weights w over the a_s columns of
the gathered payload, so the pairwise tree over slots sums [h*w | w]
together: the softmax denominator falls out of the tree for free.
Layer-2 rows (bf16, 256B): [h2 64 | a_s2 | a_d2 | pad]; w overwrites a_d2.

Softmax uses exp(leaky_relu(a_s+a_d)) without max-subtraction (logits are
O(10), no overflow), normalized once per node: out = U / max(s,tiny).

Two NEFF launches: NEFF1 = dense1 (full, redundant per core) + L1 edge
phase + normalize/ELU + dense2 -> per-core table2 shard (tab2loc, tile
layout). Host assembles the global table2. NEFF2 = L2 edge phase (a_d2 read
straight from this core's own tab2loc, no node gather) -> output shard.
Host unpermutes.
"""
import numpy as np
import ml_dtypes

N = 50000
E = 800000
IN = 128
HID = 64
HEADS = 3
OUT = 64
NCORE = 8
P = 128
NBLK = (N + P - 1) // P            # 391
NBLKP = ((NBLK + NCORE - 1) // NCORE) * NCORE  # 392
T = NBLKP // NCORE                 # 49 tiles per core
SENT = N                           # sentinel table row
NROW = N + 1
BASE = 32768                       # gather base row (signed int16 trick)
E1 = 256                           # L1 row elems (bf16)
E2 = 128                           # L2 row elems (bf16)
NEG_SLOPE = 0.2
F1 = HEADS * HID                   # 192
F1W = 196                          # [h*w 192 | w 3 | junk 1] tree width (L1, even)
F2W = OUT + 2                      # 66: [h2*w | a_s junk | w] width (L2)
CHUNK = 8                          # NEFF1 slots per dma_gather call
CHUNK2 = 16                        # NEFF2 (double desc ring via bigger scratch)
SMAX1 = 48                         # max slots (j * d_g) per NEFF1 group
SMAX2 = 96                         # NEFF2 groups (gen-bound: fewer, bigger)

bf16 = ml_dtypes.bfloat16

# interleaved feature order: row position i=(c*3+h) holds feature f=h*64+c
_POS = np.arange(F1)
PERM = (_POS % HEADS) * HID + (_POS // HEADS)   # row pos i -> original feature


def _pack_idx(rows_flat):
    """rows_flat int64[nidx] (table rows, nidx%128==0) -> int16 [128, nidx//16]
    wrap-16 layout replicated across the 8 groups of 16 partitions."""
    v = (rows_flat - BASE).astype(np.int16)
    cid = len(v) // 16
    block = v.reshape(cid, 16).T
    return np.tile(block, (8, 1))


def _chunk_spans(S, dg, chunk=CHUNK):
    """(s0, s1) spans of the dma_gather calls over S flat slots. A call must
    not END on a slot-0 (self-row) position: slot 0 holds the tile's own node
    row (a_d source) and its int16 idx can be negative -> trailing-drop.
    Calls never exceed `chunk` slots: chunk*128 descs must fit the ring."""
    spans = []
    s0 = 0
    while s0 < S:
        s1 = min(s0 + chunk, S)
        if s1 < S and (s1 - 1) % dg == 0:
            s1 -= 1
        spans.append((s0, s1))
        s0 = s1
    return spans


def make_groups(d_t, smax):
    """[(t0, j, d_g)] with j*d_g <= smax; d_t is sorted desc so d_g=d_t[t0]."""
    groups = []
    t = 0
    while t < T:
        dg = int(d_t[t])
        j = 1
        while t + j < T and (j + 1) * dg <= smax:
            j += 1
        groups.append((t, j, dg))
        t += j
    return groups


def preprocess(edge_index):
    src = np.concatenate([edge_index[0].astype(np.int64), np.arange(N, dtype=np.int64)])
    dst = np.concatenate([edge_index[1].astype(np.int64), np.arange(N, dtype=np.int64)])
    deg = np.bincount(dst, minlength=N)              # includes the self-loop
    dege = deg - 1                                   # true edges (self excluded)
    order = np.argsort(-deg, kind="stable")          # node processing order
    eorder = np.argsort(dst, kind="stable")
    esrc = src[eorder]                               # edge srcs grouped by dst
    eptr = np.zeros(N + 1, np.int64)
    eptr[1:] = np.cumsum(deg)

    # Node placement. Nodes sorted by degree desc. Partition 127 of every
    # (core, tile) must hold a LOW-degree node: the gather drops trailing-
    # negative indices per chunked call, so p127's call-final slots must be
    # sentinel pads. Reserve the globally lowest-degree nodes for the p127
    # positions; fill the rest of each tile interleaved across cores.
    n127 = T * NCORE                       # 392 p127 positions
    main = order[:N - n127]                # desc
    tail = order[N - n127:]                # lowest degrees, desc within
    nid = np.full((NCORE, T, P), SENT, np.int64)
    MP = P - 1                             # main slots per (core,tile)
    for t in range(T):
        blk = main[t * NCORE * MP: (t + 1) * NCORE * MP]
        for c in range(NCORE):
            sl = blk[c::NCORE]
            nid[c, t, :len(sl)] = sl       # p 0..126 (short for last tile)
        tb = tail[t * NCORE: (t + 1) * NCORE]
        for c in range(NCORE):
            if c < len(tb):
                nid[c, t, P - 1] = tb[c]

    d_t = []
    for t in range(T):
        dmax = 2
        d127 = 0
        for c in range(NCORE):
            for p in range(P - 1):
                if nid[c, t, p] != SENT:
                    dmax = max(dmax, int(deg[nid[c, t, p]]))
                    break                  # interleaved desc: p=0 is max
            if nid[c, t, P - 1] != SENT:
                d127 = max(d127, int(deg[nid[c, t, P - 1]]))
        # p127 needs deg <= d - (number of call boundaries hitting its tile)
        d_t.append(max(dmax, d127 + (d127 + CHUNK - 1) // CHUNK + 2, 2))
    d_t = np.array(d_t, np.int64)

    groups1 = make_groups(d_t, SMAX1)
    groups2 = make_groups(d_t, SMAX2)
    CE1 = 8 * sum(j * dg for _, j, dg in groups1)
    CE2 = 8 * sum(j * dg for _, j, dg in groups2)

    def edge_grid(c, t0, j, dg, chunk=CHUNK):
        # [P, j, dg] slot grid: slot 0 of each tile row = the node's OWN row
        # (self-loop; doubles as the a_d source), remaining slots = edges.
        S = j * dg
        bnd = {s1 - 1 for (_, s1) in _chunk_spans(S, dg, chunk)}
        rows = np.full((P, j, dg), SENT, np.int64)
        for jj in range(j):
            t = t0 + jj
            p127_ok = [s for s in range(1, dg) if (jj * dg + s) not in bnd]
            for p in range(P):
                n = nid[c, t, p]
                if n == SENT:
                    continue
                rows[p, jj, 0] = n
                k = int(dege[n])
                if p == P - 1:
                    assert k <= len(p127_ok), (t, k, len(p127_ok))
                    rows[p, jj, p127_ok[:k]] = esrc[eptr[n]:eptr[n] + k]
                else:
                    rows[p, jj, 1:1 + k] = esrc[eptr[n]:eptr[n] + k]
        # flat slot major, partition minor
        return rows.transpose(1, 2, 0).reshape(-1)

    idxe1 = np.zeros((NCORE, P, CE1), np.int16)
    idxe2 = np.zeros((NCORE, P, CE2), np.int16)
    for c in range(NCORE):
        idxe1[c] = np.concatenate(
            [_pack_idx(edge_grid(c, t0, j, dg)) for (t0, j, dg) in groups1], axis=1)
        idxe2[c] = np.concatenate(
            [_pack_idx(edge_grid(c, t0, j, dg, CHUNK2)) for (t0, j, dg) in groups2], axis=1)

    return dict(order=order, nid=nid, d_t=d_t, groups1=groups1,
                groups2=groups2, idxe1=idxe1, idxe2=idxe2, deg=deg,
                esrc=esrc, eptr=eptr)


def host_weights(x, W1, att_src1, att_dst1, b1, W2, att_src2, att_dst2, b2):
    W1s = np.stack([W1[:, h * HID:(h + 1) * HID] @ att_src1[h] for h in range(HEADS)], 1)  # [128,3]
    W1d = np.stack([W1[:, h * HID:(h + 1) * HID] @ att_dst1[h] for h in range(HEADS)], 1)
    W1e = np.concatenate([W1[:, PERM], W1s, W1d], axis=1)           # [128,198]
    W2e = np.concatenate([W2, (W2 @ att_src2[0])[:, None], (W2 @ att_dst2[0])[:, None]], 1)  # [192,66]
    W2e = W2e[PERM, :]                                               # rows in interleaved order
    xT = np.zeros((IN, NBLK * P), np.float32)
    xT[:, :N] = x.T
    sent1 = np.zeros(E1, bf16)
    sent1[192:195] = bf16(-1e30)            # a_s = -inf-ish (bf16)
    b1i = b1[PERM].astype(bf16)
    return dict(xT=xT.astype(bf16), W1e=W1e.astype(bf16),
                W2e1=W2e[:128].astype(bf16), W2e2=W2e[128:].astype(bf16),
                sent1=sent1.reshape(1, E1), b1i=np.tile(b1i, (P, 1)),
                b2b=np.tile(b2.astype(np.float32), (P, 1)))


# ---------------------------------------------------------------- emulation
def _bf(a):
    return a.astype(bf16).astype(np.float32)


def _tree(vals):
    """pairwise bf16 tree-sum over axis 1 of [P, n, f]."""
    vs = [vals[:, i] for i in range(vals.shape[1])]
    while len(vs) > 1:
        nxt = []
        for i in range(0, len(vs) - 1, 2):
            nxt.append(_bf(vs[i] + vs[i + 1]))
        if len(vs) % 2:
            nxt.append(vs[-1])
        vs = nxt
    return vs[0]


def emulate(x, edge_index, W1, att_src1, att_dst1, b1, W2, att_src2, att_dst2, b2):
    """numpy emulation of the device pipeline (bf16 rounding where the HW has it)."""
    pre = preprocess(edge_index)
    hw = host_weights(x, W1, att_src1, att_dst1, b1, W2, att_src2, att_dst2, b2)
    nid = pre["nid"]
    groups1, groups2 = pre["groups1"], pre["groups2"]
    deg, esrc, eptr = pre["deg"], pre["esrc"], pre["eptr"]

    xTf = hw["xT"].astype(np.float32)[:, :N]
    W1ef = hw["W1e"].astype(np.float32)
    H = xTf.T @ W1ef                      # [N, 198] f32
    Hb = _bf(H)                           # table rows all bf16
    tab_h = np.vstack([Hb[:, :F1], np.zeros((1, F1), np.float32)])
    tab_as = np.vstack([Hb[:, 192:195], np.full((1, HEADS), float(bf16(-1e30)), np.float32)])
    tab_ad = np.vstack([Hb[:, 195:198], np.zeros((1, HEADS), np.float32)])

    tab2_h = np.zeros((NROW, OUT), np.float32)
    tab2_as = np.zeros((NROW, 1), np.float32)
    tab2_as[SENT] = -1e30
    ad2_loc = np.zeros((NCORE, T, P), np.float32)
    W2ef = np.concatenate([hw["W2e1"], hw["W2e2"]], 0).astype(np.float32)
    b1f = hw["b1i"].astype(np.float32)[0]

    def edge_rows(c, t, dg):
        rows = np.full((P, dg), SENT, np.int64)
        for p in range(P):
            n = nid[c, t, p]
            if n != SENT:
                k = int(deg[n])
                rows[p, :k] = esrc[eptr[n]:eptr[n] + k]
        return rows

    for c in range(NCORE):
        for (t0, j, dg) in groups1:
            for jj in range(j):
                t = t0 + jj
                rows = edge_rows(c, t, dg)
                g_h = tab_h[rows]                       # [P,dg,192]
                g_as = tab_as[rows]                     # [P,dg,3]
                a_d = tab_ad[nid[c, t]]                 # [P,3]
                e = g_as + a_d[:, None, :]
                e = np.maximum(e, NEG_SLOPE * e)
                wb = _bf(np.exp(e))                     # bf16 w
                prod = _bf(g_h * wb.repeat(HID, axis=2).reshape(P, dg, HEADS, HID)
                           .transpose(0, 1, 3, 2).reshape(P, dg, F1))
                tre = _tree(np.concatenate([prod, wb], axis=2))   # [P,195]
                U, s = tre[:, :F1], tre[:, F1:]
                rcb = _bf(1.0 / np.maximum(s, 1e-30))   # [P,3]
                h1 = _bf(U * np.repeat(rcb[:, None, :], HID, 1).reshape(P, F1))
                h1 = _bf(h1 + b1f)
                h1 = _bf(np.maximum(h1, 0) + _bf(np.exp(np.minimum(h1, 0))) - 1)
                out2 = _bf(h1) @ W2ef                    # [P,66] f32 accum
                out2 = _bf(out2)
                valid = nid[c, t] != SENT
                nn = nid[c, t][valid]
                tab2_h[nn] = out2[valid, :OUT]
                tab2_as[nn, 0] = out2[valid, OUT]
                ad2_loc[c, t] = out2[:, OUT + 1]

    outp = np.zeros((N, OUT), np.float32)
    b2f = hw["b2b"][0]
    for c in range(NCORE):
        for (t0, j, dg) in groups2:
            for jj in range(j):
                t = t0 + jj
                rows = edge_rows(c, t, dg)
                g_h = tab2_h[rows]                      # [P,dg,64]
                g_as = tab2_as[rows]                    # [P,dg,1]
                a_d = ad2_loc[c, t][:, None]            # [P,1]
                e = g_as + a_d[:, None, :]
                e = np.maximum(e, NEG_SLOPE * e)
                wb = _bf(np.exp(e))
                prod = _bf(g_h * wb)
                tre = _tree(np.concatenate([prod, wb], axis=2))  # [P,65]
                U2, s = tre[:, :OUT], tre[:, OUT:]
                r = 1.0 / np.maximum(s, 1e-30)
                o = U2 * r + b2f
                valid = nid[c, t] != SENT
                outp[nid[c, t][valid]] = o[valid]
    return outp


# ---------------------------------------------------------------- bass build
def _tree_levels(nc, mybir, dt, trp, cur_tile, j, d, fw, estride):
    """pairwise tree over d slots within each of j tiles.
    cur_tile: [P, j*d*estride] (level 0, estride=E1/E2) -> returns [P, j*fw]."""
    cur, n, stride = cur_tile, d, estride
    lvl = 0
    while n > 1:
        half, odd = n // 2, n % 2
        dst = trp.tile([P, j * (half + odd) * fw], dt.bfloat16,
                       tag=f"tr{1 + (lvl % 2)}")
        src = cur[:, :j * n * stride].rearrange("p (j s e) -> p j s e", j=j, e=stride)
        nc.vector.tensor_tensor(
            out=dst[:].rearrange("p (j s f) -> p j s f", j=j, f=fw)[:, :, :half],
            in0=src[:, :, 0:2 * half:2, :fw],
            in1=src[:, :, 1:2 * half:2, :fw],
            op=mybir.AluOpType.add)
        if odd:
            nc.vector.tensor_copy(
                out=dst[:].rearrange("p (j s f) -> p j s f", j=j, f=fw)[:, :, half],
                in_=src[:, :, n - 1, :fw])
        cur, n, stride = dst, half + odd, fw
        lvl += 1
    return cur


def _build_neff1(groups, pay_bufs=3, trp_bufs=2, wp_bufs=3, hp_bufs=2):
    import concourse.bacc as bacc
    import concourse.mybir as mybir
    import concourse.tile as tile
    from concourse.masks import make_identity

    dt = mybir.dt
    nc = bacc.Bacc(num_swdge_queues=4)
    CE = 8 * sum(j * dg for _, j, dg in groups)
    xT = nc.dram_tensor("xT", [IN, NBLK * P], dt.bfloat16, kind="ExternalInput")
    W1e = nc.dram_tensor("W1e", [IN, 198], dt.bfloat16, kind="ExternalInput")
    W2e1 = nc.dram_tensor("W2e1", [128, 66], dt.bfloat16, kind="ExternalInput")
    W2e2 = nc.dram_tensor("W2e2", [64, 66], dt.bfloat16, kind="ExternalInput")
    b1i = nc.dram_tensor("b1i", [P, F1], dt.bfloat16, kind="ExternalInput")
    sent1 = nc.dram_tensor("sent1", [1, E1], dt.bfloat16, kind="ExternalInput")
    idxe = nc.dram_tensor("idxe", [P, CE], dt.int16, kind="ExternalInput")
    tab2loc = nc.dram_tensor("tab2loc", [T * P, 68], dt.bfloat16, kind="ExternalOutput")
    tab1 = nc.dram_tensor("tab1", [NROW, E1], dt.bfloat16)

    FQ = [0]

    def q():
        FQ[0] = (FQ[0] + 1) % 4
        return FQ[0]

    with tile.TileContext(nc) as tc:
        with tc.tile_pool(name="const", bufs=1) as cp, \
             tc.tile_pool(name="xp", bufs=3) as xp, \
             tc.tile_pool(name="rowp", bufs=3) as rowp, \
             tc.tile_pool(name="gp", bufs=pay_bufs) as gpool, \
             tc.tile_pool(name="wp", bufs=wp_bufs) as wpool, \
             tc.tile_pool(name="trp", bufs=trp_bufs) as trp, \
             tc.tile_pool(name="hp", bufs=hp_bufs) as hpool, \
             tc.tile_pool(name="psA", bufs=2, space="PSUM") as psA, \
             tc.tile_pool(name="psB", bufs=2, space="PSUM") as psB:

            w1_sb = cp.tile([IN, 198], dt.bfloat16)
            nc.sync.dma_start(out=w1_sb[:], in_=W1e[:, :])
            w2a_sb = cp.tile([128, 66], dt.bfloat16)
            nc.sync.dma_start(out=w2a_sb[:], in_=W2e1[:, :])
            w2b_sb = cp.tile([64, 66], dt.bfloat16)
            nc.sync.dma_start(out=w2b_sb[:], in_=W2e2[:, :])
            b1_sb = cp.tile([P, F1], dt.bfloat16)
            nc.sync.dma_start(out=b1_sb[:], in_=b1i[:, :])
            ide = cp.tile([P, P], dt.bfloat16)
            make_identity(nc, ide[:])
            ie_sb = cp.tile([P, CE], dt.int16)
            nc.sync.dma_start(out=ie_sb[:], in_=idxe[:, :])
            sent_sb = cp.tile([1, E1], dt.bfloat16)
            nc.sync.dma_start(out=sent_sb[:], in_=sent1[:, :])
            nc.sync.dma_start(out=tab1[SENT:SENT + 1, :], in_=sent_sb[:])

            # ---------------- dense1: tab1 rows for all nodes
            XC = 8
            for ch in range((NBLK + XC - 1) // XC):
                ntile = min(XC, NBLK - ch * XC)
                xch = xp.tile([P, XC * P], dt.bfloat16, tag="x")
                nc.sync.dma_start(
                    out=xch[:, :ntile * P],
                    in_=xT[:, ch * XC * P: ch * XC * P + ntile * P])
                rt = rowp.tile([P, XC * E1], dt.bfloat16, tag="rt")
                k = 0
                while k < ntile:
                    kk = min(2, ntile - k)       # pair two matmuls per psum bank
                    pt = psA.tile([P, 396], dt.float32, tag="d1")
                    for i in range(kk):
                        nc.tensor.matmul(pt[:, i * 198:(i + 1) * 198],
                                         lhsT=xch[:, (k + i) * P:(k + i + 1) * P],
                                         rhs=w1_sb[:], start=True, stop=True)
                    dst = rt[:, k * E1:(k + kk) * E1] \
                        .rearrange("p (k e) -> p k e", e=E1)[:, :, :198]
                    srcv = pt[:, :kk * 198].rearrange("p (k e) -> p k e", e=198)
                    if (k // 2) % 2 == 0:
                        nc.scalar.activation(dst, srcv,
                                             mybir.ActivationFunctionType.Copy)
                    else:
                        nc.vector.tensor_copy(out=dst, in_=srcv)
                    k += kk
                full = min(ntile * P, N - ch * XC * P)
                dst = tab1[ch * XC * P: ch * XC * P + full, :]
                if full == ntile * P:
                    dst_v = dst.rearrange("(k p) e -> p k e", p=P)
                    src_v = rt[:, :ntile * E1].rearrange("p (k e) -> p k e", e=E1)
                    nc.sync.dma_start(out=dst_v, in_=src_v)
                else:
                    for k in range(ntile):
                        nt = ch * XC + k
                        nrows = min(P, N - nt * P)
                        if nrows > 0:
                            nc.sync.dma_start(
                                out=tab1[nt * P: nt * P + nrows, :],
                                in_=rt[:nrows, k * E1:(k + 1) * E1])

            # ---------------- edge phase L1 + dense2, group-batched
            tab_lo = tab1[BASE:, :]
            off_e = 0
            for (t0, j, dg) in groups:
                S = j * dg
                pt = gpool.tile([P, S * E1], dt.bfloat16, tag="pay")
                for (s0, s1) in _chunk_spans(S, dg):
                    nc.gpsimd.dma_gather(
                        out_ap=pt[:, s0 * E1:s1 * E1].rearrange("p (s e) -> p s e", e=E1),
                        in_ap=tab_lo,
                        idxs_ap=ie_sb[:, off_e + 8 * s0: off_e + 8 * s1],
                        num_idxs=(s1 - s0) * P, num_idxs_reg=(s1 - s0) * P,
                        elem_size=E1, single_packet=False, queue_num=q())
                off_e += 8 * S

                pe_ = pt[:].rearrange("p (j d e) -> p j d e", j=j, e=E1)
                a_s = pe_[:, :, :, 192:195]
                a_d = pe_[:, :, 0, 195:198] \
                    .unsqueeze(2).to_broadcast([P, j, dg, 3])
                et = wpool.tile([P, S * 3], dt.float32, tag="e")
                nc.vector.tensor_tensor(
                    out=et[:].rearrange("p (j d h) -> p j d h", j=j, h=3),
                    in0=a_s, in1=a_d, op=mybir.AluOpType.add)
                et2 = wpool.tile([P, S * 3], dt.float32, tag="e2")
                nc.vector.scalar_tensor_tensor(
                    out=et2[:], in0=et[:], scalar=NEG_SLOPE, in1=et[:],
                    op0=mybir.AluOpType.mult, op1=mybir.AluOpType.max)
                # exp -> bf16 w written over the a_s columns of the payload
                pse = pt[:].rearrange("p (s e) -> p s e", e=E1)
                nc.scalar.activation(pse[:, :, 192:195],
                                     et2[:].rearrange("p (s h) -> p s h", h=3),
                                     mybir.ActivationFunctionType.Exp)
                # h *= w   (interleaved (c,h): every operand innermost stride 1)
                h_view = pse[:, :, :F1].rearrange("p s (c h) -> p s c h", h=3)
                w_view = pse[:, :, 192:195].unsqueeze(2).to_broadcast([P, S, HID, 3])
                nc.vector.tensor_tensor(out=h_view, in0=h_view, in1=w_view,
                                        op=mybir.AluOpType.mult)
                # pairwise tree over slots: sums [h*w | w | junk] -> [P, j*196]
                cur = _tree_levels(nc, mybir, dt, trp, pt, j, dg, F1W, E1)
                curv = cur[:].rearrange("p (j f) -> p j f", f=F1W)
                # normalize + bias + ELU
                sc = wpool.tile([P, j * 3], dt.float32, tag="sc")
                nc.vector.tensor_scalar_max(
                    out=sc[:].rearrange("p (j h) -> p j h", h=3),
                    in0=curv[:, :, F1:F1 + 3], scalar1=1e-30)
                rc = wpool.tile([P, j * 3], dt.float32, tag="rc")
                nc.vector.reciprocal(out=rc[:], in_=sc[:])
                rcb = wpool.tile([P, j * 3], dt.bfloat16, tag="rcb")
                nc.vector.tensor_copy(out=rcb[:], in_=rc[:])
                h1 = hpool.tile([P, j * F1], dt.bfloat16, tag="h1")
                nc.vector.tensor_tensor(
                    out=h1[:].rearrange("p (j c h) -> p j c h", j=j, h=3),
                    in0=curv[:, :, :F1].rearrange("p j (c h) -> p j c h", h=3),
                    in1=rcb[:].rearrange("p (j h) -> p j h", h=3)
                        .unsqueeze(2).to_broadcast([P, j, HID, 3]),
                    op=mybir.AluOpType.mult)
                nc.vector.tensor_tensor(
                    out=h1[:].rearrange("p (j f) -> p j f", f=F1),
                    in0=h1[:].rearrange("p (j f) -> p j f", f=F1),
                    in1=b1_sb[:].unsqueeze(1).to_broadcast([P, j, F1]),
                    op=mybir.AluOpType.add)
                a1 = hpool.tile([P, j * F1], dt.bfloat16, tag="tmin")
                nc.scalar.activation(a1[:], h1[:],
                                     mybir.ActivationFunctionType.Relu, scale=-1.0)
                texp = hpool.tile([P, j * F1], dt.bfloat16, tag="texp")
                nc.scalar.activation(texp[:], a1[:],
                                     mybir.ActivationFunctionType.Exp, scale=-1.0)
                rp = hpool.tile([P, j * F1], dt.bfloat16, tag="rp")
                nc.scalar.activation(rp[:], h1[:], mybir.ActivationFunctionType.Relu)
                h1e = hpool.tile([P, j * F1], dt.bfloat16, tag="h1e")
                nc.vector.scalar_tensor_tensor(
                    out=h1e[:], in0=rp[:], scalar=-1.0, in1=texp[:],
                    op0=mybir.AluOpType.add, op1=mybir.AluOpType.add)
                # dense2 per tile (PE) into a group output row
                r2 = rowp.tile([P, j * 68], dt.bfloat16, tag="r2")
                for jj in range(j):
                    hsl = h1e[:, jj * F1:(jj + 1) * F1]
                    tp1 = psB.tile([P, P], dt.bfloat16, tag="tp1")
                    nc.tensor.transpose(tp1[:], hsl[:, :P], ide[:])
                    tp2 = psB.tile([64, P], dt.bfloat16, tag="tp2")
                    nc.tensor.transpose(tp2[:], hsl[:, P:F1], ide[:])
                    hT1 = hpool.tile([P, P], dt.bfloat16, tag="hT1")
                    nc.scalar.activation(hT1[:], tp1[:],
                                         mybir.ActivationFunctionType.Copy)
                    hT2 = hpool.tile([64, P], dt.bfloat16, tag="hT2")
                    nc.scalar.activation(hT2[:], tp2[:],
                                         mybir.ActivationFunctionType.Copy)
                    o2 = psA.tile([P, 66], dt.float32, tag="o2")
                    nc.tensor.matmul(o2[:], lhsT=hT1[:], rhs=w2a_sb[:], start=True, stop=False)
                    nc.tensor.matmul(o2[:], lhsT=hT2[:], rhs=w2b_sb[:], start=False, stop=True)
                    nc.scalar.activation(r2[:, jj * 68:jj * 68 + 66], o2[:, :66],
                                         mybir.ActivationFunctionType.Copy)
                nc.sync.dma_start(
                    out=tab2loc[t0 * P:(t0 + j) * P, :].rearrange("(j p) e -> p j e", p=P),
                    in_=r2[:].rearrange("p (j e) -> p j e", e=68))
    nc.compile()
    return nc


def _build_neff2(groups, pay_bufs=4, trp_bufs=2, wp_bufs=3):
    import concourse.bacc as bacc
    import concourse.mybir as mybir
    import concourse.tile as tile

    dt = mybir.dt
    nc = bacc.Bacc(num_swdge_queues=4, dynamic_dma_scratch_size=32768)
    CE = 8 * sum(j * dg for _, j, dg in groups)
    tab2 = nc.dram_tensor("tab2", [NROW, E2], dt.bfloat16, kind="ExternalInput")
    t2l = nc.dram_tensor("t2l", [T * P, 68], dt.bfloat16, kind="ExternalInput")
    idxe = nc.dram_tensor("idxe", [P, CE], dt.int16, kind="ExternalInput")
    b2b = nc.dram_tensor("b2b", [P, OUT], dt.float32, kind="ExternalInput")
    out2 = nc.dram_tensor("out2", [T * P, OUT], dt.float32, kind="ExternalOutput")

    FQ = [0]

    def q():
        FQ[0] = (FQ[0] + 1) % 4
        return FQ[0]

    with tile.TileContext(nc) as tc:
        with tc.tile_pool(name="const", bufs=1) as cp, \
             tc.tile_pool(name="gp", bufs=pay_bufs) as gpool, \
             tc.tile_pool(name="wp", bufs=wp_bufs) as wpool, \
             tc.tile_pool(name="trp", bufs=trp_bufs) as trp, \
             tc.tile_pool(name="op", bufs=3) as opool:
            ie_sb = cp.tile([P, CE], dt.int16)
            nc.sync.dma_start(out=ie_sb[:], in_=idxe[:, :])
            b2_sb = cp.tile([P, OUT], dt.float32)
            nc.sync.dma_start(out=b2_sb[:], in_=b2b[:, :])
            t2l_sb = cp.tile([P, T * 68], dt.bfloat16)
            nc.sync.dma_start(out=t2l_sb[:].rearrange("p (t e) -> p t e", e=68),
                              in_=t2l[:, :].rearrange("(t p) e -> p t e", p=P))

            tab_lo = tab2[BASE:, :]
            off_e = 0
            for (t0, j, dg) in groups:
                S = j * dg
                pt = gpool.tile([P, S * E2], dt.bfloat16, tag="pay")
                for (s0, s1) in _chunk_spans(S, dg, CHUNK2):
                    nc.gpsimd.dma_gather(
                        out_ap=pt[:, s0 * E2:s1 * E2].rearrange("p (s e) -> p s e", e=E2),
                        in_ap=tab_lo,
                        idxs_ap=ie_sb[:, off_e + 8 * s0: off_e + 8 * s1],
                        num_idxs=(s1 - s0) * P, num_idxs_reg=(s1 - s0) * P,
                        elem_size=E2, single_packet=False, queue_num=q())
                off_e += 8 * S

                pjde = pt[:].rearrange("p (j d e) -> p j d e", j=j, e=E2)
                a_s = pjde[:, :, :, 64:65]
                a_d = t2l_sb[:].rearrange("p (t e) -> p t e", e=68)[:, t0:t0 + j, 65:66] \
                    .unsqueeze(2).to_broadcast([P, j, dg, 1])
                et = wpool.tile([P, S], dt.float32, tag="e")
                nc.vector.tensor_tensor(
                    out=et[:].rearrange("p (j d) -> p j d", j=j).unsqueeze(3),
                    in0=a_s, in1=a_d, op=mybir.AluOpType.add)
                et2 = wpool.tile([P, S], dt.float32, tag="e2")
                nc.vector.scalar_tensor_tensor(
                    out=et2[:], in0=et[:], scalar=NEG_SLOPE, in1=et[:],
                    op0=mybir.AluOpType.mult, op1=mybir.AluOpType.max)
                pse = pt[:].rearrange("p (s e) -> p s e", e=E2)
                # w over the a_d2 column (65); tree over cols 0:66 sums
                # [h2*w | a_s junk | w]
                nc.scalar.activation(pse[:, :, 65:66],
                                     et2[:].unsqueeze(2),
                                     mybir.ActivationFunctionType.Exp)
                h_view = pse[:, :, :OUT]
                w_view = pse[:, :, 65:66].to_broadcast([P, S, OUT])
                nc.vector.tensor_tensor(out=h_view, in0=h_view, in1=w_view,
                                        op=mybir.AluOpType.mult)
                cur = _tree_levels(nc, mybir, dt, trp, pt, j, dg, F2W, E2)
                curv = cur[:].rearrange("p (j f) -> p j f", f=F2W)
                sc = wpool.tile([P, j], dt.float32, tag="sc")
                nc.vector.tensor_scalar_max(
                    out=sc[:].unsqueeze(2), in0=curv[:, :, 65:66], scalar1=1e-30)
                rc = wpool.tile([P, j], dt.float32, tag="rc")
                nc.vector.reciprocal(out=rc[:], in_=sc[:])
                ot = opool.tile([P, j * OUT], dt.float32, tag="ot")
                nc.vector.tensor_tensor(
                    out=ot[:].rearrange("p (j f) -> p j f", f=OUT),
                    in0=curv[:, :, :OUT],
                    in1=rc[:].unsqueeze(2).to_broadcast([P, j, OUT]),
                    op=mybir.AluOpType.mult)
                nc.vector.tensor_tensor(
                    out=ot[:].rearrange("p (j f) -> p j f", f=OUT),
                    in0=ot[:].rearrange("p (j f) -> p j f", f=OUT),
                    in1=b2_sb[:].unsqueeze(1).to_broadcast([P, j, OUT]),
                    op=mybir.AluOpType.add)
                nc.sync.dma_start(
                    out=out2[t0 * P:(t0 + j) * P, :].rearrange("(j p) e -> p j e", p=P),
                    in_=ot[:].rearrange("p (j e) -> p j e", e=OUT))
    nc.compile()
    return nc


# ---------------------------------------------------------------- kernel
def kernel(x, edge_index, W1, att_src1, att_dst1, b1, W2, att_src2, att_dst2, b2,
           _emulate=False, _timing=None):
    x = np.asarray(x, np.float32)
    edge_index = np.asarray(edge_index)
    W1 = np.asarray(W1, np.float32)
    att_src1 = np.asarray(att_src1, np.float32)
    att_dst1 = np.asarray(att_dst1, np.float32)
    b1 = np.asarray(b1, np.float32)
    W2 = np.asarray(W2, np.float32)
    att_src2 = np.asarray(att_src2, np.float32)
    att_dst2 = np.asarray(att_dst2, np.float32)
    b2 = np.asarray(b2, np.float32)

    if _emulate:
        return emulate(x, edge_index, W1, att_src1, att_dst1, b1,
                       W2, att_src2, att_dst2, b2)

    from concourse.bass_utils import run_bass_kernel_spmd
    import time as _time

    def _run(nc, maps, trace):
        for attempt in range(3):
            try:
                return run_bass_kernel_spmd(nc, maps, core_ids=list(range(NCORE)),
                                            trace=trace and attempt == 0)
            except Exception:
                if attempt == 2:
                    raise
                _time.sleep(45)

    pre = preprocess(edge_index)
    hw = host_weights(x, W1, att_src1, att_dst1, b1, W2, att_src2, att_dst2, b2)
    nid = pre["nid"]

    trace = _timing is not None

    # ---- NEFF1
    nc1 = _build_neff1(pre["groups1"])
    maps1 = [dict(xT=hw["xT"], W1e=hw["W1e"], W2e1=hw["W2e1"], W2e2=hw["W2e2"],
                  b1i=hw["b1i"], sent1=hw["sent1"],
                  idxe=pre["idxe1"][c]) for c in range(NCORE)]
    res1 = _run(nc1, maps1, trace)

    # host: assemble global table2
    tab2 = np.zeros((NROW, E2), bf16)
    tab2[SENT, 64] = bf16(-1e30)
    locs = []
    for c in range(NCORE):
        loc = res1.results[c]["tab2loc"]           # [T*P, 68] bf16
        locs.append(loc)
        nn = nid[c].reshape(-1)                    # [T*P]
        valid = nn != SENT
        tab2[nn[valid], :66] = loc[valid][:, :66]

    # ---- NEFF2
    nc2 = _build_neff2(pre["groups2"])
    maps2 = [dict(tab2=tab2, t2l=locs[c], idxe=pre["idxe2"][c],
                  b2b=hw["b2b"]) for c in range(NCORE)]
    res2 = _run(nc2, maps2, trace)

    out = np.zeros((N, OUT), np.float32)
    for c in range(NCORE):
        o = res2.results[c]["out2"]
        nn = nid[c].reshape(-1)
        valid = nn != SENT
        out[nn[valid]] = o[valid]

    if _timing is not None:
        _timing["neff1_ns"] = res1.exec_time_ns
        _timing["neff2_ns"] = res2.exec_time_ns
    return out


# revision 24
# speedup vs baseline: 1.0021x; 1.0021x over previous
"""2-layer GAT (PyG-style) on TRN2, 8 NeuronCores.

Strategy: degree-bucketed node layout. Nodes sorted by in-degree (desc) and
dealt round-robin across the 8 cores into 49 tiles/core of 128 nodes each
(so every core's tile t has near-identical max degree -> one shared SPMD
schedule). Tiles are batched into GROUPS of j consecutive tiles sharing a
common (padded) degree d_g, so every per-edge DVE op runs once per group
instead of once per tile. Each group grid is [128 node-partitions x j x d_g]
edge slots padded with a sentinel row whose a_s = -1e30 (=> w = 0). Per-edge
source rows are fetched with dma_gather (GPSIMD bulk gather; signed int16
indices against mid-table base row 32768; chunked ~8 slots/call to keep the
descriptor rings draining across the 4 SWDGE queues).

Layer-1 table rows (bf16, 512B): [h (192, (c,h)-interleaved) | a_s 3 | a_d 3
| pad], all bf16. The exp() writes the edge weights w over the a_s columns of
the gathered payload, so the pairwise tree over slots sums [h*w | w]
together: the softmax denominator falls out of the tree for free.
Layer-2 rows (bf16, 256B): [h2 64 | a_s2 | a_d2 | pad]; w overwrites a_d2.

Softmax uses exp(leaky_relu(a_s+a_d)) without max-subtraction (logits are
O(10), no overflow), normalized once per node: out = U / max(s,tiny).

Two NEFF launches: NEFF1 = dense1 (full, redundant per core) + L1 edge
phase + normalize/ELU + dense2 -> per-core table2 shard (tab2loc, tile
layout). Host assembles the global table2. NEFF2 = L2 edge phase (a_d2 read
straight from this core's own tab2loc, no node gather) -> output shard.
Host unpermutes.
"""
import numpy as np
import ml_dtypes

N = 50000
E = 800000
IN = 128
HID = 64
HEADS = 3
OUT = 64
NCORE = 8
P = 128
NBLK = (N + P - 1) // P            # 391
NBLKP = ((NBLK + NCORE - 1) // NCORE) * NCORE  # 392
T = NBLKP // NCORE                 # 49 tiles per core
SENT = N                           # sentinel table row
NROW = N + 1
BASE = 32768                       # gather base row (signed int16 trick)
E1 = 256                           # L1 row elems (bf16)
E2 = 128                           # L2 row elems (bf16)
NEG_SLOPE = 0.2
F1 = HEADS * HID                   # 192
F1W = 196                          # [h*w 192 | w 3 | junk 1] tree width (L1, even)
F2W = OUT + 2                      # 66: [h2*w | a_s junk | w] width (L2)
CHUNK = 8                          # NEFF1 slots per dma_gather call
CHUNK2 = 8                         # NEFF2 slots per dma_gather call
SMAX1 = 48                         # max slots (j * d_g) per NEFF1 group
SMAX2 = 96                         # NEFF2 groups (gen-bound: fewer, bigger)

bf16 = ml_dtypes.bfloat16

# interleaved feature order: row position i=(c*3+h) holds feature f=h*64+c
_POS = np.arange(F1)
PERM = (_POS % HEADS) * HID + (_POS // HEADS)   # row pos i -> original feature


def _pack_idx(rows_flat):
    """rows_flat int64[nidx] (table rows, nidx%128==0) -> int16 [128, nidx//16]
    wrap-16 layout replicated across the 8 groups of 16 partitions."""
    v = (rows_flat - BASE).astype(np.int16)
    cid = len(v) // 16
    block = v.reshape(cid, 16).T
    return np.tile(block, (8, 1))


def _chunk_spans(S, dg, chunk=CHUNK):
    """(s0, s1) spans of the dma_gather calls over S flat slots. A call must
    not END on a slot-0 (self-row) position: slot 0 holds the tile's own node
    row (a_d source) and its int16 idx can be negative -> trailing-drop.
    Calls never exceed `chunk` slots: chunk*128 descs must fit the ring."""
    spans = []
    s0 = 0
    while s0 < S:
        s1 = min(s0 + chunk, S)
        if s1 < S and (s1 - 1) % dg == 0:
            s1 -= 1
        spans.append((s0, s1))
        s0 = s1
    return spans


def make_groups(d_t, smax):
    """[(t0, j, d_g)] with j*d_g <= smax; d_t is sorted desc so d_g=d_t[t0]."""
    groups = []
    t = 0
    while t < T:
        dg = int(d_t[t])
        j = 1
        while t + j < T and (j + 1) * dg <= smax:
            j += 1
        groups.append((t, j, dg))
        t += j
    return groups


def preprocess(edge_index):
    src = np.concatenate([edge_index[0].astype(np.int64), np.arange(N, dtype=np.int64)])
    dst = np.concatenate([edge_index[1].astype(np.int64), np.arange(N, dtype=np.int64)])
    deg = np.bincount(dst, minlength=N)              # includes the self-loop
    dege = deg - 1                                   # true edges (self excluded)
    order = np.argsort(-deg, kind="stable")          # node processing order
    eorder = np.argsort(dst, kind="stable")
    esrc = src[eorder]                               # edge srcs grouped by dst
    eptr = np.zeros(N + 1, np.int64)
    eptr[1:] = np.cumsum(deg)

    # Node placement. Nodes sorted by degree desc. Partition 127 of every
    # (core, tile) must hold a LOW-degree node: the gather drops trailing-
    # negative indices per chunked call, so p127's call-final slots must be
    # sentinel pads. Reserve the globally lowest-degree nodes for the p127
    # positions; fill the rest of each tile interleaved across cores.
    n127 = T * NCORE                       # 392 p127 positions
    main = order[:N - n127]                # desc
    tail = order[N - n127:]                # lowest degrees, desc within
    nid = np.full((NCORE, T, P), SENT, np.int64)
    MP = P - 1                             # main slots per (core,tile)
    for t in range(T):
        blk = main[t * NCORE * MP: (t + 1) * NCORE * MP]
        for c in range(NCORE):
            sl = blk[c::NCORE]
            nid[c, t, :len(sl)] = sl       # p 0..126 (short for last tile)
        tb = tail[t * NCORE: (t + 1) * NCORE]
        for c in range(NCORE):
            if c < len(tb):
                nid[c, t, P - 1] = tb[c]

    d_t = []
    for t in range(T):
        dmax = 2
        d127 = 0
        for c in range(NCORE):
            for p in range(P - 1):
                if nid[c, t, p] != SENT:
                    dmax = max(dmax, int(deg[nid[c, t, p]]))
                    break                  # interleaved desc: p=0 is max
            if nid[c, t, P - 1] != SENT:
                d127 = max(d127, int(deg[nid[c, t, P - 1]]))
        # p127 needs deg <= d - (number of call boundaries hitting its tile)
        d_t.append(max(dmax, d127 + (d127 + CHUNK - 1) // CHUNK + 2, 2))
    d_t = np.array(d_t, np.int64)

    groups1 = make_groups(d_t, SMAX1)
    groups2 = make_groups(d_t, SMAX2)
    CE1 = 8 * sum(j * dg for _, j, dg in groups1)
    CE2 = 8 * sum(j * dg for _, j, dg in groups2)

    def edge_grid(c, t0, j, dg, chunk=CHUNK):
        # [P, j, dg] slot grid: slot 0 of each tile row = the node's OWN row
        # (self-loop; doubles as the a_d source), remaining slots = edges.
        S = j * dg
        bnd = {s1 - 1 for (_, s1) in _chunk_spans(S, dg, chunk)}
        rows = np.full((P, j, dg), SENT, np.int64)
        for jj in range(j):
            t = t0 + jj
            p127_ok = [s for s in range(1, dg) if (jj * dg + s) not in bnd]
            for p in range(P):
                n = nid[c, t, p]
                if n == SENT:
                    continue
                rows[p, jj, 0] = n
                k = int(dege[n])
                if p == P - 1:
                    assert k <= len(p127_ok), (t, k, len(p127_ok))
                    rows[p, jj, p127_ok[:k]] = esrc[eptr[n]:eptr[n] + k]
                else:
                    rows[p, jj, 1:1 + k] = esrc[eptr[n]:eptr[n] + k]
        # flat slot major, partition minor
        return rows.transpose(1, 2, 0).reshape(-1)

    idxe1 = np.zeros((NCORE, P, CE1), np.int16)
    idxe2 = np.zeros((NCORE, P, CE2), np.int16)
    for c in range(NCORE):
        idxe1[c] = np.concatenate(
            [_pack_idx(edge_grid(c, t0, j, dg)) for (t0, j, dg) in groups1], axis=1)
        idxe2[c] = np.concatenate(
            [_pack_idx(edge_grid(c, t0, j, dg, CHUNK2)) for (t0, j, dg) in groups2], axis=1)

    return dict(order=order, nid=nid, d_t=d_t, groups1=groups1,
                groups2=groups2, idxe1=idxe1, idxe2=idxe2, deg=deg,
                esrc=esrc, eptr=eptr)


def host_weights(x, W1, att_src1, att_dst1, b1, W2, att_src2, att_dst2, b2):
    W1s = np.stack([W1[:, h * HID:(h + 1) * HID] @ att_src1[h] for h in range(HEADS)], 1)  # [128,3]
    W1d = np.stack([W1[:, h * HID:(h + 1) * HID] @ att_dst1[h] for h in range(HEADS)], 1)
    W1e = np.concatenate([W1[:, PERM], W1s, W1d], axis=1)           # [128,198]
    W2e = np.concatenate([W2, (W2 @ att_src2[0])[:, None], (W2 @ att_dst2[0])[:, None]], 1)  # [192,66]
    W2e = W2e[PERM, :]                                               # rows in interleaved order
    xT = np.zeros((IN, NBLK * P), np.float32)
    xT[:, :N] = x.T
    sent1 = np.zeros(E1, bf16)
    sent1[192:195] = bf16(-1e30)            # a_s = -inf-ish (bf16)
    b1i = b1[PERM].astype(bf16)
    return dict(xT=xT.astype(bf16), W1e=W1e.astype(bf16),
                W2e1=W2e[:128].astype(bf16), W2e2=W2e[128:].astype(bf16),
                sent1=sent1.reshape(1, E1), b1i=np.tile(b1i, (P, 1)),
                b2b=np.tile(b2.astype(np.float32), (P, 1)))


# ---------------------------------------------------------------- emulation
def _bf(a):
    return a.astype(bf16).astype(np.float32)


def _tree(vals):
    """pairwise bf16 tree-sum over axis 1 of [P, n, f]."""
    vs = [vals[:, i] for i in range(vals.shape[1])]
    while len(vs) > 1:
        nxt = []
        for i in range(0, len(vs) - 1, 2):
            nxt.append(_bf(vs[i] + vs[i + 1]))
        if len(vs) % 2:
            nxt.append(vs[-1])
        vs = nxt
    return vs[0]


def emulate(x, edge_index, W1, att_src1, att_dst1, b1, W2, att_src2, att_dst2, b2):
    """numpy emulation of the device pipeline (bf16 rounding where the HW has it)."""
    pre = preprocess(edge_index)
    hw = host_weights(x, W1, att_src1, att_dst1, b1, W2, att_src2, att_dst2, b2)
    nid = pre["nid"]
    groups1, groups2 = pre["groups1"], pre["groups2"]
    deg, esrc, eptr = pre["deg"], pre["esrc"], pre["eptr"]

    xTf = hw["xT"].astype(np.float32)[:, :N]
    W1ef = hw["W1e"].astype(np.float32)
    H = xTf.T @ W1ef                      # [N, 198] f32
    Hb = _bf(H)                           # table rows all bf16
    tab_h = np.vstack([Hb[:, :F1], np.zeros((1, F1), np.float32)])
    tab_as = np.vstack([Hb[:, 192:195], np.full((1, HEADS), float(bf16(-1e30)), np.float32)])
    tab_ad = np.vstack([Hb[:, 195:198], np.zeros((1, HEADS), np.float32)])

    tab2_h = np.zeros((NROW, OUT), np.float32)
    tab2_as = np.zeros((NROW, 1), np.float32)
    tab2_as[SENT] = -1e30
    ad2_loc = np.zeros((NCORE, T, P), np.float32)
    W2ef = np.concatenate([hw["W2e1"], hw["W2e2"]], 0).astype(np.float32)
    b1f = hw["b1i"].astype(np.float32)[0]

    def edge_rows(c, t, dg):
        rows = np.full((P, dg), SENT, np.int64)
        for p in range(P):
            n = nid[c, t, p]
            if n != SENT:
                k = int(deg[n])
                rows[p, :k] = esrc[eptr[n]:eptr[n] + k]
        return rows

    for c in range(NCORE):
        for (t0, j, dg) in groups1:
            for jj in range(j):
                t = t0 + jj
                rows = edge_rows(c, t, dg)
                g_h = tab_h[rows]                       # [P,dg,192]
                g_as = tab_as[rows]                     # [P,dg,3]
                a_d = tab_ad[nid[c, t]]                 # [P,3]
                e = g_as + a_d[:, None, :]
                e = np.maximum(e, NEG_SLOPE * e)
                wb = _bf(np.exp(e))                     # bf16 w
                prod = _bf(g_h * wb.repeat(HID, axis=2).reshape(P, dg, HEADS, HID)
                           .transpose(0, 1, 3, 2).reshape(P, dg, F1))
                tre = _tree(np.concatenate([prod, wb], axis=2))   # [P,195]
                U, s = tre[:, :F1], tre[:, F1:]
                rcb = _bf(1.0 / np.maximum(s, 1e-30))   # [P,3]
                h1 = _bf(U * np.repeat(rcb[:, None, :], HID, 1).reshape(P, F1))
                h1 = _bf(h1 + b1f)
                h1 = _bf(np.maximum(h1, 0) + _bf(np.exp(np.minimum(h1, 0))) - 1)
                out2 = _bf(h1) @ W2ef                    # [P,66] f32 accum
                out2 = _bf(out2)
                valid = nid[c, t] != SENT
                nn = nid[c, t][valid]
                tab2_h[nn] = out2[valid, :OUT]
                tab2_as[nn, 0] = out2[valid, OUT]
                ad2_loc[c, t] = out2[:, OUT + 1]

    outp = np.zeros((N, OUT), np.float32)
    b2f = hw["b2b"][0]
    for c in range(NCORE):
        for (t0, j, dg) in groups2:
            for jj in range(j):
                t = t0 + jj
                rows = edge_rows(c, t, dg)
                g_h = tab2_h[rows]                      # [P,dg,64]
                g_as = tab2_as[rows]                    # [P,dg,1]
                a_d = ad2_loc[c, t][:, None]            # [P,1]
                e = g_as + a_d[:, None, :]
                e = np.maximum(e, NEG_SLOPE * e)
                wb = _bf(np.exp(e))
                prod = _bf(g_h * wb)
                tre = _tree(np.concatenate([prod, wb], axis=2))  # [P,65]
                U2, s = tre[:, :OUT], tre[:, OUT:]
                r = 1.0 / np.maximum(s, 1e-30)
                o = U2 * r + b2f
                valid = nid[c, t] != SENT
                outp[nid[c, t][valid]] = o[valid]
    return outp


# ---------------------------------------------------------------- bass build
def _tree_levels(nc, mybir, dt, trp, cur_tile, j, d, fw, estride):
    """pairwise tree over d slots within each of j tiles.
    cur_tile: [P, j*d*estride] (level 0, estride=E1/E2) -> returns [P, j*fw]."""
    cur, n, stride = cur_tile, d, estride
    lvl = 0
    while n > 1:
        half, odd = n // 2, n % 2
        dst = trp.tile([P, j * (half + odd) * fw], dt.bfloat16,
                       tag=f"tr{1 + (lvl % 2)}")
        src = cur[:, :j * n * stride].rearrange("p (j s e) -> p j s e", j=j, e=stride)
        nc.vector.tensor_tensor(
            out=dst[:].rearrange("p (j s f) -> p j s f", j=j, f=fw)[:, :, :half],
            in0=src[:, :, 0:2 * half:2, :fw],
            in1=src[:, :, 1:2 * half:2, :fw],
            op=mybir.AluOpType.add)
        if odd:
            nc.vector.tensor_copy(
                out=dst[:].rearrange("p (j s f) -> p j s f", j=j, f=fw)[:, :, half],
                in_=src[:, :, n - 1, :fw])
        cur, n, stride = dst, half + odd, fw
        lvl += 1
    return cur


def _build_neff1(groups, pay_bufs=3, trp_bufs=2, wp_bufs=3, hp_bufs=2):
    import concourse.bacc as bacc
    import concourse.mybir as mybir
    import concourse.tile as tile
    from concourse.masks import make_identity

    dt = mybir.dt
    nc = bacc.Bacc(num_swdge_queues=4)
    CE = 8 * sum(j * dg for _, j, dg in groups)
    xT = nc.dram_tensor("xT", [IN, NBLK * P], dt.bfloat16, kind="ExternalInput")
    W1e = nc.dram_tensor("W1e", [IN, 198], dt.bfloat16, kind="ExternalInput")
    W2e1 = nc.dram_tensor("W2e1", [128, 66], dt.bfloat16, kind="ExternalInput")
    W2e2 = nc.dram_tensor("W2e2", [64, 66], dt.bfloat16, kind="ExternalInput")
    b1i = nc.dram_tensor("b1i", [P, F1], dt.bfloat16, kind="ExternalInput")
    sent1 = nc.dram_tensor("sent1", [1, E1], dt.bfloat16, kind="ExternalInput")
    idxe = nc.dram_tensor("idxe", [P, CE], dt.int16, kind="ExternalInput")
    tab2loc = nc.dram_tensor("tab2loc", [T * P, 68], dt.bfloat16, kind="ExternalOutput")
    tab1 = nc.dram_tensor("tab1", [NROW, E1], dt.bfloat16)

    FQ = [0]

    def q():
        FQ[0] = (FQ[0] + 1) % 4
        return FQ[0]

    with tile.TileContext(nc) as tc:
        with tc.tile_pool(name="const", bufs=1) as cp, \
             tc.tile_pool(name="xp", bufs=3) as xp, \
             tc.tile_pool(name="rowp", bufs=3) as rowp, \
             tc.tile_pool(name="gp", bufs=pay_bufs) as gpool, \
             tc.tile_pool(name="wp", bufs=wp_bufs) as wpool, \
             tc.tile_pool(name="trp", bufs=trp_bufs) as trp, \
             tc.tile_pool(name="hp", bufs=hp_bufs) as hpool, \
             tc.tile_pool(name="psA", bufs=2, space="PSUM") as psA, \
             tc.tile_pool(name="psB", bufs=2, space="PSUM") as psB:

            w1_sb = cp.tile([IN, 198], dt.bfloat16)
            nc.sync.dma_start(out=w1_sb[:], in_=W1e[:, :])
            w2a_sb = cp.tile([128, 66], dt.bfloat16)
            nc.sync.dma_start(out=w2a_sb[:], in_=W2e1[:, :])
            w2b_sb = cp.tile([64, 66], dt.bfloat16)
            nc.sync.dma_start(out=w2b_sb[:], in_=W2e2[:, :])
            b1_sb = cp.tile([P, F1], dt.bfloat16)
            nc.sync.dma_start(out=b1_sb[:], in_=b1i[:, :])
            ide = cp.tile([P, P], dt.bfloat16)
            make_identity(nc, ide[:])
            ie_sb = cp.tile([P, CE], dt.int16)
            nc.sync.dma_start(out=ie_sb[:], in_=idxe[:, :])
            sent_sb = cp.tile([1, E1], dt.bfloat16)
            nc.sync.dma_start(out=sent_sb[:], in_=sent1[:, :])
            nc.sync.dma_start(out=tab1[SENT:SENT + 1, :], in_=sent_sb[:])

            # ---------------- dense1: tab1 rows for all nodes
            XC = 8
            for ch in range((NBLK + XC - 1) // XC):
                ntile = min(XC, NBLK - ch * XC)
                xch = xp.tile([P, XC * P], dt.bfloat16, tag="x")
                nc.sync.dma_start(
                    out=xch[:, :ntile * P],
                    in_=xT[:, ch * XC * P: ch * XC * P + ntile * P])
                rt = rowp.tile([P, XC * E1], dt.bfloat16, tag="rt")
                k = 0
                while k < ntile:
                    kk = min(2, ntile - k)       # pair two matmuls per psum bank
                    pt = psA.tile([P, 396], dt.float32, tag="d1")
                    for i in range(kk):
                        nc.tensor.matmul(pt[:, i * 198:(i + 1) * 198],
                                         lhsT=xch[:, (k + i) * P:(k + i + 1) * P],
                                         rhs=w1_sb[:], start=True, stop=True)
                    dst = rt[:, k * E1:(k + kk) * E1] \
                        .rearrange("p (k e) -> p k e", e=E1)[:, :, :198]
                    srcv = pt[:, :kk * 198].rearrange("p (k e) -> p k e", e=198)
                    if (k // 2) % 2 == 0:
                        nc.scalar.activation(dst, srcv,
                                             mybir.ActivationFunctionType.Copy)
                    else:
                        nc.vector.tensor_copy(out=dst, in_=srcv)
                    k += kk
                full = min(ntile * P, N - ch * XC * P)
                dst = tab1[ch * XC * P: ch * XC * P + full, :]
                if full == ntile * P:
                    dst_v = dst.rearrange("(k p) e -> p k e", p=P)
                    src_v = rt[:, :ntile * E1].rearrange("p (k e) -> p k e", e=E1)
                    nc.sync.dma_start(out=dst_v, in_=src_v)
                else:
                    for k in range(ntile):
                        nt = ch * XC + k
                        nrows = min(P, N - nt * P)
                        if nrows > 0:
                            nc.sync.dma_start(
                                out=tab1[nt * P: nt * P + nrows, :],
                                in_=rt[:nrows, k * E1:(k + 1) * E1])

            # ---------------- edge phase L1 + dense2, group-batched
            tab_lo = tab1[BASE:, :]
            off_e = 0
            for (t0, j, dg) in groups:
                S = j * dg
                pt = gpool.tile([P, S * E1], dt.bfloat16, tag="pay")
                for (s0, s1) in _chunk_spans(S, dg):
                    nc.gpsimd.dma_gather(
                        out_ap=pt[:, s0 * E1:s1 * E1].rearrange("p (s e) -> p s e", e=E1),
                        in_ap=tab_lo,
                        idxs_ap=ie_sb[:, off_e + 8 * s0: off_e + 8 * s1],
                        num_idxs=(s1 - s0) * P, num_idxs_reg=(s1 - s0) * P,
                        elem_size=E1, single_packet=True, queue_num=q())
                off_e += 8 * S

                pe_ = pt[:].rearrange("p (j d e) -> p j d e", j=j, e=E1)
                a_s = pe_[:, :, :, 192:195]
                a_d = pe_[:, :, 0, 195:198] \
                    .unsqueeze(2).to_broadcast([P, j, dg, 3])
                et = wpool.tile([P, S * 3], dt.float32, tag="e")
                nc.vector.tensor_tensor(
                    out=et[:].rearrange("p (j d h) -> p j d h", j=j, h=3),
                    in0=a_s, in1=a_d, op=mybir.AluOpType.add)
                et2 = wpool.tile([P, S * 3], dt.float32, tag="e2")
                nc.vector.scalar_tensor_tensor(
                    out=et2[:], in0=et[:], scalar=NEG_SLOPE, in1=et[:],
                    op0=mybir.AluOpType.mult, op1=mybir.AluOpType.max)
                # exp -> bf16 w written over the a_s columns of the payload
                pse = pt[:].rearrange("p (s e) -> p s e", e=E1)
                nc.scalar.activation(pse[:, :, 192:195],
                                     et2[:].rearrange("p (s h) -> p s h", h=3),
                                     mybir.ActivationFunctionType.Exp)
                # h *= w   (interleaved (c,h): every operand innermost stride 1)
                h_view = pse[:, :, :F1].rearrange("p s (c h) -> p s c h", h=3)
                w_view = pse[:, :, 192:195].unsqueeze(2).to_broadcast([P, S, HID, 3])
                nc.vector.tensor_tensor(out=h_view, in0=h_view, in1=w_view,
                                        op=mybir.AluOpType.mult)
                # pairwise tree over slots: sums [h*w | w | junk] -> [P, j*196]
                cur = _tree_levels(nc, mybir, dt, trp, pt, j, dg, F1W, E1)
                curv = cur[:].rearrange("p (j f) -> p j f", f=F1W)
                # normalize + bias + ELU
                sc = wpool.tile([P, j * 3], dt.float32, tag="sc")
                nc.vector.tensor_scalar_max(
                    out=sc[:].rearrange("p (j h) -> p j h", h=3),
                    in0=curv[:, :, F1:F1 + 3], scalar1=1e-30)
                rc = wpool.tile([P, j * 3], dt.float32, tag="rc")
                nc.vector.reciprocal(out=rc[:], in_=sc[:])
                rcb = wpool.tile([P, j * 3], dt.bfloat16, tag="rcb")
                nc.vector.tensor_copy(out=rcb[:], in_=rc[:])
                h1 = hpool.tile([P, j * F1], dt.bfloat16, tag="h1")
                nc.vector.tensor_tensor(
                    out=h1[:].rearrange("p (j c h) -> p j c h", j=j, h=3),
                    in0=curv[:, :, :F1].rearrange("p j (c h) -> p j c h", h=3),
                    in1=rcb[:].rearrange("p (j h) -> p j h", h=3)
                        .unsqueeze(2).to_broadcast([P, j, HID, 3]),
                    op=mybir.AluOpType.mult)
                nc.vector.tensor_tensor(
                    out=h1[:].rearrange("p (j f) -> p j f", f=F1),
                    in0=h1[:].rearrange("p (j f) -> p j f", f=F1),
                    in1=b1_sb[:].unsqueeze(1).to_broadcast([P, j, F1]),
                    op=mybir.AluOpType.add)
                a1 = hpool.tile([P, j * F1], dt.bfloat16, tag="tmin")
                nc.scalar.activation(a1[:], h1[:],
                                     mybir.ActivationFunctionType.Relu, scale=-1.0)
                texp = hpool.tile([P, j * F1], dt.bfloat16, tag="texp")
                nc.scalar.activation(texp[:], a1[:],
                                     mybir.ActivationFunctionType.Exp, scale=-1.0)
                rp = hpool.tile([P, j * F1], dt.bfloat16, tag="rp")
                nc.scalar.activation(rp[:], h1[:], mybir.ActivationFunctionType.Relu)
                h1e = hpool.tile([P, j * F1], dt.bfloat16, tag="h1e")
                nc.vector.scalar_tensor_tensor(
                    out=h1e[:], in0=rp[:], scalar=-1.0, in1=texp[:],
                    op0=mybir.AluOpType.add, op1=mybir.AluOpType.add)
                # dense2 per tile (PE) into a group output row
                r2 = rowp.tile([P, j * 68], dt.bfloat16, tag="r2")
                for jj in range(j):
                    hsl = h1e[:, jj * F1:(jj + 1) * F1]
                    tp1 = psB.tile([P, P], dt.bfloat16, tag="tp1")
                    nc.tensor.transpose(tp1[:], hsl[:, :P], ide[:])
                    tp2 = psB.tile([64, P], dt.bfloat16, tag="tp2")
                    nc.tensor.transpose(tp2[:], hsl[:, P:F1], ide[:])
                    hT1 = hpool.tile([P, P], dt.bfloat16, tag="hT1")
                    nc.scalar.activation(hT1[:], tp1[:],
                                         mybir.ActivationFunctionType.Copy)
                    hT2 = hpool.tile([64, P], dt.bfloat16, tag="hT2")
                    nc.scalar.activation(hT2[:], tp2[:],
                                         mybir.ActivationFunctionType.Copy)
                    o2 = psA.tile([P, 66], dt.float32, tag="o2")
                    nc.tensor.matmul(o2[:], lhsT=hT1[:], rhs=w2a_sb[:], start=True, stop=False)
                    nc.tensor.matmul(o2[:], lhsT=hT2[:], rhs=w2b_sb[:], start=False, stop=True)
                    nc.scalar.activation(r2[:, jj * 68:jj * 68 + 66], o2[:, :66],
                                         mybir.ActivationFunctionType.Copy)
                nc.sync.dma_start(
                    out=tab2loc[t0 * P:(t0 + j) * P, :].rearrange("(j p) e -> p j e", p=P),
                    in_=r2[:].rearrange("p (j e) -> p j e", e=68))
    nc.compile()
    return nc


def _build_neff2(groups, pay_bufs=4, trp_bufs=2, wp_bufs=3):
    import concourse.bacc as bacc
    import concourse.mybir as mybir
    import concourse.tile as tile

    dt = mybir.dt
    nc = bacc.Bacc(num_swdge_queues=4)
    CE = 8 * sum(j * dg for _, j, dg in groups)
    tab2 = nc.dram_tensor("tab2", [NROW, E2], dt.bfloat16, kind="ExternalInput")
    t2l = nc.dram_tensor("t2l", [T * P, 68], dt.bfloat16, kind="ExternalInput")
    idxe = nc.dram_tensor("idxe", [P, CE], dt.int16, kind="ExternalInput")
    b2b = nc.dram_tensor("b2b", [P, OUT], dt.float32, kind="ExternalInput")
    out2 = nc.dram_tensor("out2", [T * P, OUT], dt.float32, kind="ExternalOutput")

    FQ = [0]

    def q():
        FQ[0] = (FQ[0] + 1) % 4
        return FQ[0]

    with tile.TileContext(nc) as tc:
        with tc.tile_pool(name="const", bufs=1) as cp, \
             tc.tile_pool(name="gp", bufs=pay_bufs) as gpool, \
             tc.tile_pool(name="wp", bufs=wp_bufs) as wpool, \
             tc.tile_pool(name="trp", bufs=trp_bufs) as trp, \
             tc.tile_pool(name="op", bufs=3) as opool:
            ie_sb = cp.tile([P, CE], dt.int16)
            nc.sync.dma_start(out=ie_sb[:], in_=idxe[:, :])
            b2_sb = cp.tile([P, OUT], dt.float32)
            nc.sync.dma_start(out=b2_sb[:], in_=b2b[:, :])
            t2l_sb = cp.tile([P, T * 68], dt.bfloat16)
            nc.sync.dma_start(out=t2l_sb[:].rearrange("p (t e) -> p t e", e=68),
                              in_=t2l[:, :].rearrange("(t p) e -> p t e", p=P))

            tab_lo = tab2[BASE:, :]
            off_e = 0
            for (t0, j, dg) in groups:
                S = j * dg
                pt = gpool.tile([P, S * E2], dt.bfloat16, tag="pay")
                for (s0, s1) in _chunk_spans(S, dg, CHUNK2):
                    nc.gpsimd.dma_gather(
                        out_ap=pt[:, s0 * E2:s1 * E2].rearrange("p (s e) -> p s e", e=E2),
                        in_ap=tab_lo,
                        idxs_ap=ie_sb[:, off_e + 8 * s0: off_e + 8 * s1],
                        num_idxs=(s1 - s0) * P, num_idxs_reg=(s1 - s0) * P,
                        elem_size=E2, single_packet=True, queue_num=q())
                off_e += 8 * S

                pjde = pt[:].rearrange("p (j d e) -> p j d e", j=j, e=E2)
                a_s = pjde[:, :, :, 64:65]
                a_d = t2l_sb[:].rearrange("p (t e) -> p t e", e=68)[:, t0:t0 + j, 65:66] \
                    .unsqueeze(2).to_broadcast([P, j, dg, 1])
                et = wpool.tile([P, S], dt.float32, tag="e")
                nc.vector.tensor_tensor(
                    out=et[:].rearrange("p (j d) -> p j d", j=j).unsqueeze(3),
                    in0=a_s, in1=a_d, op=mybir.AluOpType.add)
                et2 = wpool.tile([P, S], dt.float32, tag="e2")
                nc.vector.scalar_tensor_tensor(
                    out=et2[:], in0=et[:], scalar=NEG_SLOPE, in1=et[:],
                    op0=mybir.AluOpType.mult, op1=mybir.AluOpType.max)
                pse = pt[:].rearrange("p (s e) -> p s e", e=E2)
                # w over the a_d2 column (65); tree over cols 0:66 sums
                # [h2*w | a_s junk | w]
                nc.scalar.activation(pse[:, :, 65:66],
                                     et2[:].unsqueeze(2),
                                     mybir.ActivationFunctionType.Exp)
                h_view = pse[:, :, :OUT]
                w_view = pse[:, :, 65:66].to_broadcast([P, S, OUT])
                nc.vector.tensor_tensor(out=h_view, in0=h_view, in1=w_view,
                                        op=mybir.AluOpType.mult)
                cur = _tree_levels(nc, mybir, dt, trp, pt, j, dg, F2W, E2)
                curv = cur[:].rearrange("p (j f) -> p j f", f=F2W)
                sc = wpool.tile([P, j], dt.float32, tag="sc")
                nc.vector.tensor_scalar_max(
                    out=sc[:].unsqueeze(2), in0=curv[:, :, 65:66], scalar1=1e-30)
                rc = wpool.tile([P, j], dt.float32, tag="rc")
                nc.vector.reciprocal(out=rc[:], in_=sc[:])
                ot = opool.tile([P, j * OUT], dt.float32, tag="ot")
                nc.vector.tensor_tensor(
                    out=ot[:].rearrange("p (j f) -> p j f", f=OUT),
                    in0=curv[:, :, :OUT],
                    in1=rc[:].unsqueeze(2).to_broadcast([P, j, OUT]),
                    op=mybir.AluOpType.mult)
                nc.vector.tensor_tensor(
                    out=ot[:].rearrange("p (j f) -> p j f", f=OUT),
                    in0=ot[:].rearrange("p (j f) -> p j f", f=OUT),
                    in1=b2_sb[:].unsqueeze(1).to_broadcast([P, j, OUT]),
                    op=mybir.AluOpType.add)
                nc.sync.dma_start(
                    out=out2[t0 * P:(t0 + j) * P, :].rearrange("(j p) e -> p j e", p=P),
                    in_=ot[:].rearrange("p (j e) -> p j e", e=OUT))
    nc.compile()
    return nc


# ---------------------------------------------------------------- kernel
def kernel(x, edge_index, W1, att_src1, att_dst1, b1, W2, att_src2, att_dst2, b2,
           _emulate=False, _timing=None):
    x = np.asarray(x, np.float32)
    edge_index = np.asarray(edge_index)
    W1 = np.asarray(W1, np.float32)
    att_src1 = np.asarray(att_src1, np.float32)
    att_dst1 = np.asarray(att_dst1, np.float32)
    b1 = np.asarray(b1, np.float32)
    W2 = np.asarray(W2, np.float32)
    att_src2 = np.asarray(att_src2, np.float32)
    att_dst2 = np.asarray(att_dst2, np.float32)
    b2 = np.asarray(b2, np.float32)

    if _emulate:
        return emulate(x, edge_index, W1, att_src1, att_dst1, b1,
                       W2, att_src2, att_dst2, b2)

    from concourse.bass_utils import run_bass_kernel_spmd
    import time as _time

    def _run(nc, maps, trace):
        for attempt in range(3):
            try:
                return run_bass_kernel_spmd(nc, maps, core_ids=list(range(NCORE)),
                                            trace=trace and attempt == 0)
            except Exception:
                if attempt == 2:
                    raise
                _time.sleep(45)

    pre = preprocess(edge_index)
    hw = host_weights(x, W1, att_src1, att_dst1, b1, W2, att_src2, att_dst2, b2)
    nid = pre["nid"]

    trace = _timing is not None

    # ---- NEFF1
    nc1 = _build_neff1(pre["groups1"])
    maps1 = [dict(xT=hw["xT"], W1e=hw["W1e"], W2e1=hw["W2e1"], W2e2=hw["W2e2"],
                  b1i=hw["b1i"], sent1=hw["sent1"],
                  idxe=pre["idxe1"][c]) for c in range(NCORE)]
    res1 = _run(nc1, maps1, trace)

    # host: assemble global table2
    tab2 = np.zeros((NROW, E2), bf16)
    tab2[SENT, 64] = bf16(-1e30)
    locs = []
    for c in range(NCORE):
        loc = res1.results[c]["tab2loc"]           # [T*P, 68] bf16
        locs.append(loc)
        nn = nid[c].reshape(-1)                    # [T*P]
        valid = nn != SENT
        tab2[nn[valid], :66] = loc[valid][:, :66]

    # ---- NEFF2
    nc2 = _build_neff2(pre["groups2"])
    maps2 = [dict(tab2=tab2, t2l=locs[c], idxe=pre["idxe2"][c],
                  b2b=hw["b2b"]) for c in range(NCORE)]
    res2 = _run(nc2, maps2, trace)

    out = np.zeros((N, OUT), np.float32)
    for c in range(NCORE):
        o = res2.results[c]["out2"]
        nn = nid[c].reshape(-1)
        valid = nn != SENT
        out[nn[valid]] = o[valid]

    if _timing is not None:
        _timing["neff1_ns"] = res1.exec_time_ns
        _timing["neff2_ns"] = res2.exec_time_ns
    return out
